# revision 1
# baseline (speedup 1.0000x reference)
"""Trainium2 Bass kernel for a BasicTransformerBlock (self-attn + cross-attn + GEGLU FF).

Sharding: 8 cores = 2 batches x 4 sequence chunks of 1024 rows. Each core
redundantly computes LN1 + K/V projections over its batch's full 4096 rows
(position-independent, so all cores run an identical SPMD program) and
produces its own 1024-row slice of the output. No collectives.

Precision: fp32 residual stream and softmax statistics; bf16 weights and
activations for projections/FF; fp8e4m3 with DoubleRow matmuls for the
self-attention score and probability-x-V products (errors there are diluted
~100x by the fp32 residual). Softmax runs without max-subtraction (scores
are provably small at this problem's scale: |s| < ~1.1) with 1/sqrt(dh)
folded into the exp; the denominator comes free from a ones-column in V.
"""

import numpy as np
import ml_dtypes

DIM = 320
HEADS = 8
DH = 40
CTX = 768
IFF = 1280  # GEGLU inner width; proj1 width = 2*IFF
EPS = 1e-5
SCALE = DH ** -0.5
NCORES = 8
MCTX = 77
VS = 336  # V row stride (8*41 = 328 padded to %16 for DoubleRow)

BF16 = ml_dtypes.bfloat16


def _chunks(total, step=128):
    out = []
    k = 0
    while k < total:
        out.append((k, min(step, total - k)))
        k += step
    return out


DIM_CHUNKS = _chunks(DIM)    # [(0,128),(128,128),(256,64)]
CTX_CHUNKS = _chunks(CTX)    # 6 x 128


def _register_exp_op():
    """Custom DVE op: out = (in0*s0 + s1)^32 — used as exp(z) ~ (1+z/32)^32
    to offload part of the softmax exp from ACT to the vector engine."""
    import concourse.dve_ops as dve_ops
    for o in dve_ops.OPS:
        if o.name == "EXP_POLY32_ANT":
            return o
    from concourse.dve_spec import Spec, Src0, C0, C1, sq
    spec = Spec(
        body=sq(sq(sq(sq(sq(Src0 * C0 + C1))))),
        reference=lambda in0, in1, s0, s1, imm2:
            ((in0.astype(np.float32) * s0 + s1) ** 32).astype(np.float32))
    op = dve_ops.DveOp("EXP_POLY32_ANT", spec, subdim=False,
                       uops_sha={"v3": "eafb894a1d5c531b"})
    dve_ops.OPS.append(op)
    dve_ops._SUB_OPCODE_FOR_NAME[op.name] = \
        dve_ops._CUSTOM_DVE_ROW_BASE + len(dve_ops.OPS) - 1
    dve_ops.CUSTOM_DVE_SPECS[op.name] = op.spec
    return op


def build_nc(S, R, flags=()):
    """Build + compile the per-core Bass program.

    flags: subset of {"ln1_w","ln1_b","ln2_w","ln2_b","ln3_w","ln3_b",
    "a1_bo","a2_bo","ff_b2"} that are non-trivial and must be applied.
    """
    import concourse.bass as bass
    import concourse.tile as tile
    from concourse import bacc, mybir
    from concourse.masks import make_identity

    f32 = mybir.dt.float32
    bf = mybir.dt.bfloat16
    f8 = mybir.dt.float8e3
    AF = mybir.ActivationFunctionType
    OP = mybir.AluOpType
    PM = mybir.MatmulPerfMode
    flags = set(flags)

    KB = S // 128     # key blocks (self-attn)
    QT = R // 128     # q row-tiles
    QHS = R // 512    # q 512-row groups

    nc = bacc.Bacc("TRN2", target_bir_lowering=False, debug=False)

    def din(name, shape, dt=bf):
        return nc.dram_tensor(name, shape, dt, kind="ExternalInput").ap()

    xfull_d = din("xfull", [S, DIM])
    xq_d = din("xq", [R, DIM], f32)
    ctxT_d = din("ctxT", [CTX, MCTX])
    w_d = {}
    for nm, shape in [
        ("a1_Wq", [DIM, 512]), ("a1_Wk", [DIM, 512]), ("a1_Wv", [DIM, DIM]),
        ("a1_Wo", [DIM, DIM]), ("a2_Wq", [DIM, 512]), ("a2_Wk", [CTX, 512]),
        ("a2_Wv", [CTX, DIM]), ("a2_Wo", [DIM, DIM]),
        ("ff_W1", [DIM, 2 * IFF]), ("ff_W2", [IFF, DIM]),
    ]:
        w_d[nm] = din(nm, shape)
    b1_d = din("ff_b1", [2 * IFF], f32)
    vec_d = {nm: din(nm, [DIM], f32) for nm in sorted(flags)}
    out_d = nc.dram_tensor("out", [R, DIM], f32, kind="ExternalOutput").ap()

    with tile.TileContext(nc) as tc:
        import contextlib
        with contextlib.ExitStack() as est:
            persist = est.enter_context(tc.tile_pool(name="persist", bufs=1))
            work = est.enter_context(tc.tile_pool(name="work", bufs=4))
            expp = est.enter_context(tc.tile_pool(name="expp", bufs=5))
            # One PSUM pool for the whole kernel: tag "sc" = 2 x [128,1024]f32
            # (4 banks), tag "acc" = 4 x [128,512]f32 (4 banks). All other
            # PSUM tiles allocate from these tags so phases can pipeline.
            psum = est.enter_context(tc.tile_pool(name="psum", bufs=2,
                                                  space="PSUM"))

            def ps_sc(shape, dt=f32, name="sc"):
                return psum.tile(shape, dt, tag="sc", bufs=2, name=name)

            def ps_acc(shape, dt=f32, name="accp"):
                return psum.tile(shape, dt, tag="acc", bufs=4, name=name)

            ident = persist.tile([128, 128], bf, name="ident")
            make_identity(nc, ident)
            eps_t = persist.tile([128, 1], f32, name="eps_t")
            nc.vector.memset(eps_t, EPS)

            # ---- persistent activations
            h1T = persist.tile([128, 3, S], bf, name="h1T")
            Kf = persist.tile([128, 4, S], bf, name="Kf")        # 2-head blocks
            Qf = persist.tile([128, 4, R], bf, name="Qf")
            Vr = persist.tile([128, KB, VS], f8, name="Vr")
            K2f = persist.tile([128, 4, MCTX], bf, name="K2f")   # 2-head blocks
            Q2f = persist.tile([128, 4, R], bf, name="Q2f")
            V2r = persist.tile([128, VS], f8, name="V2r")
            actT = persist.tile([128, 3, R], bf, name="actT")    # hqT / h2T / h3T
            resid = persist.tile([128, QT, DIM], f32, name="resid")
            Uff = persist.tile([128, IFF // 128, R], bf, name="Uff")


            # ---- weights into SBUF, [in, out] layout chunked on partitions.
            # Loaded in stages so the critical path (xq/xfull -> LN1 -> Q/K/V)
            # is not queued behind 4.7MB of cross-attn/FF weights.
            wsb = {}

            def load_w(names):
                for nm in names:
                    chks = CTX_CHUNKS if nm in ("a2_Wk", "a2_Wv") else DIM_CHUNKS
                    width = w_d[nm].shape[1]
                    t = persist.tile([128, len(chks), width], bf, name=f"w_{nm}",
                                     uniquify=True)
                    for c, (k0, kw) in enumerate(chks):
                        nc.sync.dma_start(out=t[:kw, c, :],
                                          in_=w_d[nm][k0:k0 + kw, :])
                    wsb[nm] = t

            for t in range(QT):
                nc.sync.dma_start(out=resid[:, t, :],
                                  in_=xq_d[t * 128:(t + 1) * 128, :])
            load_w(["a1_Wq", "a1_Wk", "a1_Wv"])

            bcast = {}
            for nm in sorted(flags):
                t = persist.tile([128, DIM], f32, name=f"bc_{nm}")
                src = vec_d[nm]
                bc_ap = bass.AP(tensor=src.tensor, offset=src.offset,
                                ap=[[0, 128]] + [list(p) for p in src.ap])
                nc.gpsimd.dma_start(out=t, in_=bc_ap)
                bcast[nm] = t

            def ln_into(dst_bf, src_ap, wkey, bkey):
                stats = work.tile([128, 6], f32, tag="bnst", name="stats")
                nc.vector.bn_stats(stats, src_ap)
                mv = work.tile([128, 2], f32, tag="bnagg", name="mv")
                nc.vector.bn_aggr(mv, stats)
                rstd = work.tile([128, 1], f32, tag="rstd", name="rstd")
                nc.scalar.activation(rstd, mv[:, 1:2], AF.Sqrt, bias=eps_t, scale=1.0)
                nc.vector.reciprocal(rstd, rstd)
                nc.vector.tensor_scalar(
                    out=dst_bf, in0=src_ap, scalar1=mv[:, 0:1], scalar2=rstd,
                    op0=OP.subtract, op1=OP.mult)
                if wkey in flags:
                    nc.vector.tensor_mul(out=dst_bf, in0=dst_bf, in1=bcast[wkey])
                if bkey in flags:
                    nc.vector.tensor_add(out=dst_bf, in0=dst_bf, in1=bcast[bkey])

            def transpose_into(dstT, src_bf, col0, copy_engine="dve",
                               ps_fn=None):
                for c, (k0, kw) in enumerate(DIM_CHUNKS):
                    pt = (ps_fn or ps_sc)([128, 128], bf, name="tr_ps")
                    nc.tensor.transpose(pt[:kw, :], src_bf[:, k0:k0 + kw], ident)
                    eng = copy_engine if copy_engine != "mix" else \
                        ("act" if c % 2 == 0 else "dve")
                    if eng == "act":
                        nc.scalar.activation(dstT[:kw, c, col0:col0 + 128],
                                             pt[:kw, :], AF.Identity)
                    else:
                        nc.vector.tensor_copy(out=dstT[:kw, c, col0:col0 + 128],
                                              in_=pt[:kw, :])

            def proj_fm(dst, wt, srcT, n_lo, n_hi, chks, copy_engine="dve"):
                """Feature-major projection via stationary (padded) weight cols."""
                for g in range(4):
                    for n0 in range(n_lo, n_hi, 512):
                        nw = min(512, n_hi - n0)
                        ps = ps_acc([128, 512], name="proj_ps")
                        for c, (k0, kw) in enumerate(chks):
                            nc.tensor.matmul(
                                ps[:, :nw],
                                lhsT=wt[:kw, c, 128 * g:128 * g + 128],
                                rhs=srcT[:kw, c, n0:n0 + nw],
                                start=(c == 0), stop=(c == len(chks) - 1))
                        eng = copy_engine if copy_engine != "mix" else \
                            ("act" if (g + n0 // 512) % 2 == 0 else "dve")
                        if eng == "act":
                            nc.scalar.activation(dst[:, g, n0:n0 + nw], ps[:, :nw],
                                                 AF.Identity)
                        else:
                            nc.vector.tensor_copy(out=dst[:, g, n0:n0 + nw],
                                                  in_=ps[:, :nw])

            exp_op = _register_exp_op()

            def load_late_weights():
                load_w(["a1_Wo", "a2_Wq", "a2_Wk", "a2_Wv", "a2_Wo", "ff_W1"])
                w2 = persist.tile([128, IFF // 128, DIM], bf, name="w_ff2")
                for c in range(IFF // 128):
                    nc.sync.dma_start(out=w2[:, c, :],
                                      in_=w_d["ff_W2"][c * 128:(c + 1) * 128, :])
                b1 = persist.tile([128, (2 * IFF) // 128], f32, name="b1t")
                nc.sync.dma_start(out=b1, in_=b1_d.rearrange("(c p) -> p c", p=128))
                ctxm = persist.tile([128, len(CTX_CHUNKS), MCTX], bf, name="ctxT_sb")
                for c, (k0, kw) in enumerate(CTX_CHUNKS):
                    nc.sync.dma_start(out=ctxm[:kw, c, :], in_=ctxT_d[k0:k0 + kw, :])
                return w2, b1, ctxm

            def cross_kv():
                for g in range(4):
                    ps = ps_sc([128, 128], name="k2_ps")
                    for c, (k0, kw) in enumerate(CTX_CHUNKS):
                        nc.tensor.matmul(
                            ps[:, :MCTX],
                            lhsT=wsb["a2_Wk"][:kw, c, 128 * g:128 * g + 128],
                            rhs=ctxT_sb[:kw, c, :],
                            start=(c == 0), stop=(c == len(CTX_CHUNKS) - 1))
                    nc.vector.tensor_copy(out=K2f[:, g, :], in_=ps[:, :MCTX])
                ps = ps_acc([128, 512], name="v2_ps")
                for c, (k0, kw) in enumerate(CTX_CHUNKS):
                    nc.tensor.matmul(
                        ps[:MCTX, :DIM], lhsT=ctxT_sb[:kw, c, :],
                        rhs=wsb["a2_Wv"][:kw, c, :],
                        start=(c == 0), stop=(c == len(CTX_CHUNKS) - 1))
                nc.vector.tensor_copy(
                    out=V2r[:MCTX, 0:328].rearrange("p (h c) -> p h c", c=41)[:, :, 0:40],
                    in_=ps[:MCTX, :DIM].rearrange("p (h c) -> p h c", c=40))
                nc.vector.memset(
                    V2r[:MCTX, 0:328].rearrange("p (h c) -> p h c",
                                                c=41)[:, :, 40:41], 1.0)

            # ---- own rows first: LN1 -> hqT, Qf (so attention can start as
            # soon as the leading K/V blocks exist; xq was DMA'd first above)
            for t in range(QT):
                h = work.tile([128, DIM], bf, tag="h", bufs=6, name="hq")
                ln_into(h, resid[:, t, :], "ln1_w", "ln1_b")
                transpose_into(actT, h, t * 128, copy_engine="mix")
            proj_fm(Qf, wsb["a1_Wq"], actT, 0, R, DIM_CHUNKS)

            # ---- attn1 building blocks
            def attn1_scores_exp(q0, hp, kb):
                sc = ps_sc([128, 1024], name="sc")
                for j in range(2):
                    hh = 2 * hp + j
                    g, jj = divmod(hh, 2)
                    nc.tensor.matmul(
                        sc[:, j * 512:(j + 1) * 512],
                        lhsT=Kf[64 * jj:64 * jj + 40, g, kb * 128:(kb + 1) * 128],
                        rhs=Qf[64 * jj:64 * jj + 40, g, q0:q0 + 512],
                        start=True, stop=True)
                ep = expp.tile([128, 1024], f8, tag="ep", name="ep")
                if KB >= 8 and kb % 8 in (1, 4, 6):
                    # exp(z) ~ (1+z/32)^32 on the vector engine (softmax-
                    # invariant constant error) to offload ACT
                    nc.vector._custom_dve(exp_op, out=ep, in0=sc,
                                          s0=SCALE / 32.0, s1=1.0)
                else:
                    nc.scalar.activation(ep, sc, AF.Exp, scale=SCALE)
                return ep

            def attn1_pv(acc, hp, kb, ep):
                for j in range(2):
                    hh = 2 * hp + j
                    for qs in range(4):
                        nc.tensor.matmul(
                            acc[qs][:, 41 * hh:41 * hh + 41],
                            lhsT=ep[:, j * 512 + qs * 128:j * 512 + (qs + 1) * 128],
                            rhs=Vr[:, kb, 41 * hh:41 * hh + 41],
                            start=(kb == 0), stop=(kb == KB - 1),
                            skip_group_check=True)

            # ---- LN1 + K/V production, merged per 512-column block
            for nb in range(S // 512):
                for tt in range(4):
                    t = nb * 4 + tt
                    xt = work.tile([128, DIM], bf, tag="xt", bufs=8, name="xt")
                    nc.sync.dma_start(out=xt, in_=xfull_d[t * 128:(t + 1) * 128, :])
                    h = work.tile([128, DIM], bf, tag="h", bufs=6, name="h1")
                    ln_into(h, xt, "ln1_w", "ln1_b")
                    transpose_into(h1T, h, t * 128, copy_engine="mix")
                for g in range(4):
                    ps = ps_acc([128, 512], name="kf_ps")
                    for c, (k0, kw) in enumerate(DIM_CHUNKS):
                        nc.tensor.matmul(
                            ps,
                            lhsT=wsb["a1_Wk"][:kw, c, 128 * g:128 * g + 128],
                            rhs=h1T[:kw, c, nb * 512:(nb + 1) * 512],
                            start=(c == 0), stop=(c == len(DIM_CHUNKS) - 1))
                    if g % 2 == 0:
                        nc.scalar.activation(Kf[:, g, nb * 512:(nb + 1) * 512], ps,
                                             AF.Identity)
                    else:
                        nc.vector.tensor_copy(out=Kf[:, g, nb * 512:(nb + 1) * 512],
                                              in_=ps)
                for tt in range(4):
                    t = nb * 4 + tt
                    ps = ps_acc([128, 512], name="v_ps")
                    for c, (k0, kw) in enumerate(DIM_CHUNKS):
                        nc.tensor.matmul(
                            ps[:, :DIM],
                            lhsT=h1T[:kw, c, t * 128:(t + 1) * 128],
                            rhs=wsb["a1_Wv"][:kw, c, :],
                            start=(c == 0), stop=(c == len(DIM_CHUNKS) - 1))
                    nc.vector.tensor_copy(
                        out=Vr[:, t, 0:328].rearrange("p (h c) -> p h c",
                                                      c=41)[:, :, 0:40],
                        in_=ps[:, :DIM].rearrange("p (h c) -> p h c", c=40))
                    if t % 8 == 7 or t == KB - 1:
                        lo = t - (t % 8)
                        nc.vector.memset(
                            Vr[:, lo:t + 1, 0:328].rearrange(
                                "p b (h c) -> p b h c", c=41)[:, :, :, 40], 1.0)

            # ---- shared attention epilogue: normalize, transpose, proj, add
            def finish_attn(qh, acc, wo, bo_key):
                for qs in range(4):
                    rec = work.tile([128, HEADS], f32, tag="rec", name="rec")
                    nc.vector.reciprocal(
                        rec, acc[qs].rearrange("p (h c) -> p h c", c=41)[:, :, 40])
                    arm = work.tile([128, DIM], bf, tag="arm", name="arm")
                    rb = bass.AP(tensor=rec.tensor, offset=rec.offset,
                                 ap=[list(rec.ap[0]), [rec.ap[1][0], HEADS],
                                     [0, 40]])
                    nc.vector.tensor_mul(
                        out=arm.rearrange("p (h c) -> p h c", c=40),
                        in0=acc[qs].rearrange("p (h c) -> p h c", c=41)[:, :, 0:40],
                        in1=rb)
                    afm = work.tile([128, 3, 128], bf, tag="afm", name="afm")
                    transpose_into(afm, arm, 0, ps_fn=ps_acc)
                    po = ps_acc([128, DIM], name="po")
                    for c, (k0, kw) in enumerate(DIM_CHUNKS):
                        nc.tensor.matmul(po, lhsT=afm[:kw, c, :], rhs=wo[:kw, c, :],
                                         start=(c == 0),
                                         stop=(c == len(DIM_CHUNKS) - 1))
                    t = qh * 4 + qs
                    nc.vector.tensor_add(out=resid[:, t, :], in0=resid[:, t, :],
                                         in1=po)
                    if bo_key in flags:
                        nc.vector.tensor_add(out=resid[:, t, :], in0=resid[:, t, :],
                                             in1=bcast[bo_key])

            NMT = (2 * IFF) // 128  # 20
            # ================= per q-half: attn1 -> attn2 -> FF (pipelined)
            for qh in range(QHS):
                q0 = qh * 512
                # ---- self-attention (PV software-pipelined two tiles back)
                acc = [ps_acc([128, HEADS * 41], name=f"acc{qs}")
                       for qs in range(4)]
                pending = []
                for hp in range(HEADS // 2):
                    for kb in range(KB):
                        ep = attn1_scores_exp(q0, hp, kb)
                        pending.append((hp, kb, ep))
                        if kb % 4 == 3:
                            while len(pending) > 2:
                                attn1_pv(acc, *pending.pop(0))
                for phk in pending:
                    attn1_pv(acc, *phk)
                if qh == 0:
                    # cross-attn/FF weights + context K,V: DMA'd and computed
                    # here so they hide under attn1(qh0) instead of stalling
                    # the PE before it starts
                    w2_sb, b1t, ctxT_sb = load_late_weights()
                    cross_kv()
                finish_attn(qh, acc, wsb["a1_Wo"], "a1_bo")

                # ---- cross-attention for this q-half
                for tt in range(4):
                    t = qh * 4 + tt
                    h = work.tile([128, DIM], bf, tag="h", bufs=6, name="h2")
                    ln_into(h, resid[:, t, :], "ln2_w", "ln2_b")
                    transpose_into(actT, h, t * 128)
                proj_fm(Q2f, wsb["a2_Wq"], actT, q0, q0 + 512, DIM_CHUNKS)
                acc = [ps_acc([128, HEADS * 41], name=f"acc2_{qs}")
                       for qs in range(4)]
                p2 = []
                for hp in range(HEADS // 2):
                    sc = ps_sc([128, 1024], name="sc2")
                    for j in range(2):
                        hh = 2 * hp + j
                        g, jj = divmod(hh, 2)
                        nc.tensor.matmul(
                            sc[:MCTX, j * 512:(j + 1) * 512],
                            lhsT=K2f[64 * jj:64 * jj + 40, g, :],
                            rhs=Q2f[64 * jj:64 * jj + 40, g, q0:q0 + 512],
                            start=True, stop=True)
                    ep = expp.tile([128, 1024], f8, tag="ep2", bufs=4, name="ep2")
                    nc.scalar.activation(ep[:MCTX, :], sc[:MCTX, :], AF.Exp,
                                         scale=SCALE)
                    p2.append((hp, ep))
                for hp, ep in p2:
                    for j in range(2):
                        hh = 2 * hp + j
                        for qs in range(4):
                            nc.tensor.matmul(
                                acc[qs][:, 41 * hh:41 * hh + 41],
                                lhsT=ep[:MCTX, j * 512 + qs * 128:
                                        j * 512 + (qs + 1) * 128],
                                rhs=V2r[:MCTX, 41 * hh:41 * hh + 41],
                                start=True, stop=True, skip_group_check=True)
                finish_attn(qh, acc, wsb["a2_Wo"], "a2_bo")

                # ---- GEGLU FF for this q-half
                for tt in range(4):
                    t = qh * 4 + tt
                    h = work.tile([128, DIM], bf, tag="h", bufs=6, name="h3")
                    ln_into(h, resid[:, t, :], "ln3_w", "ln3_b")
                    transpose_into(actT, h, t * 128, copy_engine="mix")
                _order = [m for pair in zip(range(NMT // 2), range(NMT // 2, NMT))
                          for m in pair]
                for mt in _order:
                    ps = ps_acc([128, 512], name="ff1_ps")
                    for c, (k0, kw) in enumerate(DIM_CHUNKS):
                        nc.tensor.matmul(
                            ps, lhsT=wsb["ff_W1"][:kw, c, mt * 128:(mt + 1) * 128],
                            rhs=actT[:kw, c, q0:q0 + 512],
                            start=(c == 0), stop=(c == len(DIM_CHUNKS) - 1))
                    if mt < NMT // 2:
                        nc.scalar.activation(Uff[:, mt, q0:q0 + 512], ps,
                                             AF.Identity,
                                             bias=b1t[:, mt:mt + 1], scale=1.0)
                    else:
                        gl = work.tile([128, 512], bf, tag="gel", name="gel")
                        nc.scalar.activation(gl, ps, AF.Gelu,
                                             bias=b1t[:, mt:mt + 1], scale=1.0)
                        mu = mt - NMT // 2
                        nc.vector.tensor_mul(out=Uff[:, mu, q0:q0 + 512],
                                             in0=Uff[:, mu, q0:q0 + 512], in1=gl)
                for tt in range(4):
                    qs = qh * 4 + tt
                    po = ps_acc([128, DIM], name="ff2_ps")
                    for c in range(IFF // 128):
                        nc.tensor.matmul(po,
                                         lhsT=Uff[:, c, qs * 128:(qs + 1) * 128],
                                         rhs=w2_sb[:, c, :],
                                         start=(c == 0), stop=(c == IFF // 128 - 1))
                    ot = work.tile([128, DIM], f32, tag="ot", name="ot")
                    nc.vector.tensor_add(out=ot, in0=resid[:, qs, :], in1=po)
                    if "ff_b2" in flags:
                        nc.vector.tensor_add(out=ot, in0=ot, in1=bcast["ff_b2"])
                    nc.sync.dma_start(out=out_d[qs * 128:(qs + 1) * 128, :], in_=ot)

    nc.compile()
    return nc


_CACHE = {}


def _get_nc(S, R, flags):
    key = (S, R, tuple(sorted(flags)))
    if key not in _CACHE:
        _CACHE[key] = build_nc(S, R, flags)
    return _CACHE[key]


def _pad_qk8(w):
    """Self-attn Q/K weight layout for fp8 DoubleRow: per head h (g=h//4,
    m=h%4), sub i: block col 128*(2g+i) + 32*m + dk <- w col 40h + 20i + dk."""
    w = np.asarray(w)
    out = np.zeros((w.shape[0], 512), w.dtype)
    for h in range(HEADS):
        g, m = divmod(h, 4)
        for i in range(2):
            c0 = 128 * (2 * g + i) + 32 * m
            out[:, c0:c0 + 20] = w[:, DH * h + 20 * i:DH * h + 20 * i + 20]
    return out


def _pad_qk2(w):
    """Cross-attn Q/K layout: 2-head groups at partition offsets {0,64}."""
    w = np.asarray(w)
    out = np.zeros((w.shape[0], 512), w.dtype)
    for h in range(HEADS):
        g, j = divmod(h, 2)
        out[:, 128 * g + 64 * j:128 * g + 64 * j + DH] = w[:, DH * h:DH * h + DH]
    return out


def make_in_maps(x, context, ln_params, weights):
    """Host-side prep: returns (flags, in_maps, R, S, Bn)."""
    x = np.asarray(x)
    context = np.asarray(context)
    Bn = x.shape[0]
    S = x.shape[1]
    R = S * Bn // NCORES
    flags = set()
    for nm in ("ln1_w", "ln2_w", "ln3_w"):
        if not np.allclose(np.asarray(ln_params[nm]), 1.0):
            flags.add(nm)
    for nm in ("ln1_b", "ln2_b", "ln3_b", "a1_bo", "a2_bo", "ff_b2"):
        if not np.allclose(np.asarray(ln_params[nm]), 0.0):
            flags.add(nm)
    weights = dict(weights)
    weights["a1_Wq"] = _pad_qk2(weights["a1_Wq"])
    weights["a1_Wk"] = _pad_qk2(weights["a1_Wk"])
    weights["a2_Wq"] = _pad_qk2(weights["a2_Wq"])
    weights["a2_Wk"] = _pad_qk2(weights["a2_Wk"])
    shared = {nm: np.ascontiguousarray(np.asarray(w).astype(BF16))
              for nm, w in weights.items()}
    shared["ff_b1"] = np.ascontiguousarray(
        np.asarray(ln_params["ff_b1"]).astype(np.float32))
    for nm in flags:
        shared[nm] = np.ascontiguousarray(
            np.asarray(ln_params[nm]).astype(np.float32))
    xbf = np.ascontiguousarray(x.astype(BF16))
    ctxT = np.ascontiguousarray(np.asarray(context).astype(BF16).transpose(0, 2, 1))
    xf32 = np.ascontiguousarray(x.astype(np.float32))
    in_maps = []
    cpb = NCORES // Bn
    for core in range(NCORES):
        b, c = divmod(core, cpb)
        m = dict(shared)
        m["xfull"] = xbf[b]
        m["xq"] = np.ascontiguousarray(xf32[b, c * R:(c + 1) * R])
        m["ctxT"] = ctxT[b]
        in_maps.append(m)
    return flags, in_maps, R, S, Bn


def kernel(x, context, ln1_w, ln1_b, ln2_w, ln2_b, ln3_w, ln3_b,
           a1_Wq, a1_Wk, a1_Wv, a1_Wo, a1_bo,
           a2_Wq, a2_Wk, a2_Wv, a2_Wo, a2_bo,
           ff_W1, ff_b1, ff_W2, ff_b2, _trace=False):
    from concourse.bass_utils import run_bass_kernel_spmd

    weights = dict(a1_Wq=a1_Wq, a1_Wk=a1_Wk, a1_Wv=a1_Wv, a1_Wo=a1_Wo,
                   a2_Wq=a2_Wq, a2_Wk=a2_Wk, a2_Wv=a2_Wv, a2_Wo=a2_Wo,
                   ff_W1=ff_W1, ff_W2=ff_W2)
    ln_params = dict(ln1_w=ln1_w, ln1_b=ln1_b, ln2_w=ln2_w, ln2_b=ln2_b,
                     ln3_w=ln3_w, ln3_b=ln3_b, a1_bo=a1_bo, a2_bo=a2_bo,
                     ff_b1=ff_b1, ff_b2=ff_b2)
    flags, in_maps, R, S, Bn = make_in_maps(x, context, ln_params, weights)
    nc = _get_nc(S, R, flags)
    res = run_bass_kernel_spmd(nc, in_maps, core_ids=list(range(NCORES)),
                               trace=_trace)
    out = np.empty((Bn, S, DIM), np.float32)
    cpb = NCORES // Bn
    for core in range(NCORES):
        b, c = divmod(core, cpb)
        out[b, c * R:(c + 1) * R] = res.results[core]["out"]
    kernel._last_result = res
    return out

